# revision 2
# baseline (speedup 1.0000x reference)
"""Tensor-parallel causal attention block for Trainium2 (8 NeuronCores).

Shapes (hardcoded): x (2, 2048, 4096), NH=32 heads of HD=128, fp32.
Sharding: tensor-parallel over heads -- each core owns 4 heads (wq/wk/wv
column-sharded, wo row-sharded); partial outputs are summed on the host.

Per-core kernel phases:
  1. fused QKV projection (f32r matmuls), RoPE fused into PSUM eviction
     using a host-side de-interleaving weight permutation + partition-swap.
  2. attention in transposed layout: scoresT = kT_blk.T-matmul, softmax
     along the partition (k) axis with the denominator computed by a
     ones-vector matmul; causal block skipping.
  3. output projection against the row-shard of wo -> partial outT.
"""
import sys

sys.path.insert(0, "/opt/trn_rl_repo")

import numpy as np

B, S, DIM, NH, HD = 2, 2048, 4096, 32, 128
NCORES = 8
HL = NH // NCORES          # 4 heads per core
BS = B * S                 # 4096 rows
P = 128
QT = 512                   # row-tile width (matmul moving dim)
NQT = BS // QT             # 8 row tiles
KO = DIM // P              # 32 contraction chunks
SCALE = 1.0 / np.sqrt(HD)
NEG = -30000.0             # "-inf" that survives exp() as exact 0

_CACHE: dict = {}


def _hrow(h, kind):
    """Row offset of head-h q/k/v inside the (1536, BS) qkvT intermediate."""
    half, idx = divmod(h, 2)
    return half * 768 + {"q": 0, "k": 256, "v": 512}[kind] + idx * P


def _build_nc():
    import concourse.mybir as mybir
    import concourse.tile as tile
    from concourse import bacc
    from concourse.masks import make_identity

    F32 = mybir.dt.float32
    F32R = mybir.dt.float32r
    AF = mybir.ActivationFunctionType
    OP = mybir.AluOpType

    nc = bacc.Bacc(trn_type="TRN2", target_bir_lowering=False, debug=False)

    xT = nc.dram_tensor("xT", [DIM, BS], F32, kind="ExternalInput").ap()
    wqkvT = nc.dram_tensor("wqkvT", [DIM, 12 * P], F32, kind="ExternalInput").ap()
    woT = nc.dram_tensor("woT", [HL * P, DIM], F32, kind="ExternalInput").ap()
    ropeA = nc.dram_tensor("ropeA", [P, BS], F32, kind="ExternalInput").ap()
    ropeB = nc.dram_tensor("ropeB", [P, BS], F32, kind="ExternalInput").ap()
    maskT = nc.dram_tensor("maskT", [QT, QT], F32, kind="ExternalInput").ap()
    ones = nc.dram_tensor("ones", [P, 1], F32, kind="ExternalInput").ap()
    outT = nc.dram_tensor("outT", [DIM, BS], F32, kind="ExternalOutput").ap()

    xT3 = xT.rearrange("(ko p) n -> p ko n", p=P)          # (128, 32, 4096)
    wqkvT3 = wqkvT.rearrange("(ko p) c -> p ko c", p=P)    # (128, 32, 1536)
    woT3 = woT.rearrange("(kc p) m -> p kc m", p=P)        # (128, 4, 4096)
    maskT3 = maskT.rearrange("(kb p) q -> p kb q", p=P)    # (128, 4, 512)

    with tile.TileContext(nc) as tc:
        with (
            nc.allow_low_precision(reason="f32r matmul pipeline"),
            tc.tile_pool(name="const", bufs=1) as cst,
            tc.tile_pool(name="dram", bufs=1, space="DRAM") as dpool,
        ):
            qkvT_d = dpool.tile([12 * P, BS], F32R)  # (1536, 4096) intermediate

            ones_sb = cst.tile([P, 1], F32R)
            nc.sync.dma_start(ones_sb[:], ones.bitcast(F32R))
            ones_row = cst.tile([1, P], F32R)
            nc.sync.dma_start(ones_row[:], ones.rearrange("p o -> o p").bitcast(F32R))
            ident = cst.tile([P, P], F32)
            make_identity(nc, ident[:])

            # ---------------- Phase 1: fused QKV projection + RoPE ----------
            with (
                tc.tile_pool(name="p1w", bufs=1) as wpool,
                tc.tile_pool(name="p1x", bufs=4) as xpool,
                tc.tile_pool(name="rope", bufs=1) as rpool,
                tc.tile_pool(name="p1ev", bufs=3) as evpool,
                tc.tile_pool(name="p1ps", bufs=8, space="PSUM") as pspool,
            ):
                rA = rpool.tile([P, BS], F32)
                rB = rpool.tile([P, BS], F32)
                nc.sync.dma_start(rA[:], ropeA)
                nc.sync.dma_start(rB[:], ropeB)

                for half in range(2):
                    w_sb = wpool.tile([P, KO, 768], F32R, tag="w")
                    nc.sync.dma_start(
                        w_sb[:], wqkvT3[:, :, half * 768 : (half + 1) * 768].bitcast(F32R)
                    )
                    for qt in range(NQT):
                        cols = slice(qt * QT, (qt + 1) * QT)
                        psums = [
                            pspool.tile([P, QT], F32, tag="pp", name=f"pp{half}_{qt}_{i}")
                            for i in range(6)
                        ]
                        for kc in range(KO):
                            x_sb = xpool.tile([P, QT], F32R, tag="x")
                            nc.sync.dma_start(x_sb[:], xT3[:, kc, cols].bitcast(F32R))
                            for m in range(6):
                                nc.tensor.matmul(
                                    psums[m][:],
                                    w_sb[:, kc, m * P : (m + 1) * P],
                                    x_sb[:],
                                    start=(kc == 0),
                                    stop=(kc == KO - 1),
                                    skip_group_check=True,
                                )
                        for m in range(6):
                            dst = qkvT_d[
                                half * 768 + m * P : half * 768 + (m + 1) * P, cols
                            ]
                            if m < 4:  # q or k head: fused RoPE eviction
                                ev1 = evpool.tile([P, QT], F32, tag="ev1")
                                nc.vector.tensor_tensor(
                                    ev1[:], psums[m][:], rA[:, cols], OP.mult
                                )
                                ev2 = evpool.tile([P, QT], F32, tag="ev2")
                                nc.vector.tensor_tensor(
                                    ev2[:], psums[m][:], rB[:, cols], OP.mult
                                )
                                ev2s = evpool.tile([P, QT], F32, tag="ev2s")
                                nc.sync.dma_start(ev2s[0:64, :], ev2[64:128, :])
                                nc.sync.dma_start(ev2s[64:128, :], ev2[0:64, :])
                                out_t = evpool.tile([P, QT], F32R, tag="evo")
                                nc.vector.tensor_tensor(
                                    out_t[:], ev1[:], ev2s[:], OP.add
                                )
                                nc.sync.dma_start(dst, out_t[:])
                            else:  # v head: plain eviction on ACT
                                out_t = evpool.tile([P, QT], F32R, tag="evo")
                                nc.scalar.copy(out_t[:], psums[m][:])
                                nc.sync.dma_start(dst, out_t[:])

            # ---------------- Phase 2: attention ----------------------------
            with tc.tile_pool(name="att", bufs=1) as attpool:
                attnT = attpool.tile([P, HL, BS], F32R)  # 64KB/partition

                with (
                    tc.tile_pool(name="bh", bufs=2) as bhpool,
                    tc.tile_pool(name="pr", bufs=18) as prpool,
                    tc.tile_pool(name="sm", bufs=3) as smpool,
                    tc.tile_pool(name="msk", bufs=1) as mpool,
                    tc.tile_pool(name="psS", bufs=3, space="PSUM") as psS,
                    tc.tile_pool(name="psO", bufs=2, space="PSUM") as psO,
                    tc.tile_pool(name="psD", bufs=1, space="PSUM") as psD,
                    tc.tile_pool(name="psT", bufs=2, space="PSUM") as psT,
                ):
                    mask_sb = mpool.tile([P, 4, QT], F32)
                    nc.sync.dma_start(mask_sb[:], maskT3)

                    for b in range(B):
                        bcols = slice(b * S, (b + 1) * S)
                        for h in range(HL):
                            qT_sb = bhpool.tile([P, S], F32R, tag="q")
                            kT_sb = bhpool.tile([P, S], F32R, tag="k")
                            vT_sb = bhpool.tile([P, S], F32R, tag="v")
                            nc.sync.dma_start(
                                qT_sb[:], qkvT_d[_hrow(h, "q") : _hrow(h, "q") + P, bcols]
                            )
                            nc.sync.dma_start(
                                kT_sb[:], qkvT_d[_hrow(h, "k") : _hrow(h, "k") + P, bcols]
                            )
                            nc.sync.dma_start(
                                vT_sb[:], qkvT_d[_hrow(h, "v") : _hrow(h, "v") + P, bcols]
                            )
                            # transpose V into (k-rows, d) blocks
                            v_bl = bhpool.tile([P, S // P, P], F32R, tag="vb")
                            for kb in range(S // P):
                                tp = psT.tile([P, P], F32, tag="tp")
                                nc.tensor.transpose(
                                    tp[:],
                                    vT_sb[:, kb * P : (kb + 1) * P].bitcast(F32),
                                    ident[:],
                                )
                                nc.scalar.copy(v_bl[:, kb, :], tp[:])

                            for jq in range(S // QT):
                                qsl = qT_sb[:, jq * QT : (jq + 1) * QT]
                                nkb = (jq + 1) * (QT // P)
                                probs = []
                                for kb in range(nkb):
                                    sP = psS.tile([P, QT], F32, tag="sP")
                                    nc.tensor.matmul(
                                        sP[:],
                                        kT_sb[:, kb * P : (kb + 1) * P],
                                        qsl,
                                        start=True,
                                        stop=True,
                                        skip_group_check=True,
                                    )
                                    if kb >= jq * (QT // P):  # diagonal band
                                        nc.vector.tensor_tensor(
                                            sP[:],
                                            sP[:],
                                            mask_sb[:, kb - jq * (QT // P), :],
                                            OP.add,
                                        )
                                    pr = prpool.tile([P, QT], F32R, tag="pr")
                                    nc.scalar.activation(
                                        pr[:], sP[:], AF.Exp, scale=SCALE
                                    )
                                    probs.append(pr)
                                outP = psO.tile([P, QT], F32, tag="outP")
                                for kb in range(nkb):
                                    nc.tensor.matmul(
                                        outP[:],
                                        v_bl[:, kb, :],
                                        probs[kb][:],
                                        start=(kb == 0),
                                        stop=(kb == nkb - 1),
                                        skip_group_check=True,
                                    )
                                denP = psD.tile([1, QT], F32, tag="denP")
                                for kb in range(nkb):
                                    nc.tensor.matmul(
                                        denP[:],
                                        ones_sb[:],
                                        probs[kb][:],
                                        start=(kb == 0),
                                        stop=(kb == nkb - 1),
                                        skip_group_check=True,
                                    )
                                rec = smpool.tile([1, QT], F32R, tag="rec")
                                nc.vector.reciprocal(rec[:], denP[:])
                                bcP = psT.tile([P, QT], F32, tag="tp")
                                nc.tensor.matmul(
                                    bcP[:],
                                    ones_row[:],
                                    rec[:],
                                    start=True,
                                    stop=True,
                                    skip_group_check=True,
                                )
                                rb = smpool.tile([P, QT], F32, tag="rb")
                                nc.scalar.copy(rb[:], bcP[:])
                                nc.vector.tensor_tensor(
                                    attnT[:, h, b * S + jq * QT : b * S + (jq + 1) * QT],
                                    outP[:],
                                    rb[:],
                                    OP.mult,
                                )

                # ---------------- Phase 3: output projection ----------------
                with (
                    tc.tile_pool(name="p3w", bufs=1) as wpool3,
                    tc.tile_pool(name="p3ev", bufs=4) as evpool3,
                    tc.tile_pool(name="p3ps", bufs=4, space="PSUM") as ps3,
                ):
                    wo_sb = wpool3.tile([P, HL, DIM], F32R)
                    nc.sync.dma_start(wo_sb[:], woT3.bitcast(F32R))
                    for qt in range(NQT):
                        cols = slice(qt * QT, (qt + 1) * QT)
                        for m in range(DIM // P):
                            oP = ps3.tile([P, QT], F32, tag="oP")
                            for kc in range(HL):
                                nc.tensor.matmul(
                                    oP[:],
                                    wo_sb[:, kc, m * P : (m + 1) * P],
                                    attnT[:, kc, cols],
                                    start=(kc == 0),
                                    stop=(kc == HL - 1),
                                    skip_group_check=True,
                                )
                            ev = evpool3.tile([P, QT], F32, tag="oev")
                            if m % 2 == 0:
                                nc.scalar.copy(ev[:], oP[:])
                            else:
                                nc.vector.tensor_copy(out=ev[:], in_=oP[:])
                            nc.sync.dma_start(outT[m * P : (m + 1) * P, cols], ev[:])
    nc.compile()
    return nc


def _prep_inputs(x, wq, wk, wv, wo, freqs_cos, freqs_sin, mask):
    """Host-side shard prep. Returns per-core input maps."""
    x = np.asarray(x, dtype=np.float32)
    wq, wk, wv, wo = (np.asarray(a, dtype=np.float32) for a in (wq, wk, wv, wo))
    freqs_cos = np.asarray(freqs_cos, dtype=np.float32)
    freqs_sin = np.asarray(freqs_sin, dtype=np.float32)
    mask = np.asarray(mask, dtype=np.float32)

    xT = np.ascontiguousarray(x.reshape(BS, DIM).T)

    cosT = freqs_cos.T  # (64, S)
    sinT = freqs_sin.T
    ropeA = np.ascontiguousarray(
        np.tile(np.concatenate([cosT, cosT], axis=0), (1, B))
    ).astype(np.float32)
    ropeB = np.ascontiguousarray(
        np.tile(np.concatenate([sinT, -sinT], axis=0), (1, B))
    ).astype(np.float32)

    band = np.maximum(mask[:QT, :QT].T, NEG).astype(np.float32)
    band = np.ascontiguousarray(band)
    ones_col = np.ones((P, 1), dtype=np.float32)

    perm = np.concatenate([np.arange(0, HD, 2), np.arange(1, HD, 2)])

    in_maps = []
    for c in range(NCORES):
        heads = [c * HL + j for j in range(HL)]
        cols = []
        for half in range(2):
            hA, hB = heads[2 * half], heads[2 * half + 1]
            cols.append(wq[hA * HD : (hA + 1) * HD][perm].T)
            cols.append(wq[hB * HD : (hB + 1) * HD][perm].T)
            cols.append(wk[hA * HD : (hA + 1) * HD][perm].T)
            cols.append(wk[hB * HD : (hB + 1) * HD][perm].T)
            cols.append(wv[hA * HD : (hA + 1) * HD].T)
            cols.append(wv[hB * HD : (hB + 1) * HD].T)
        wqkvT = np.ascontiguousarray(np.concatenate(cols, axis=1))
        woT = np.ascontiguousarray(wo[:, c * HL * HD : (c + 1) * HL * HD].T)
        in_maps.append(
            {
                "xT": xT,
                "wqkvT": wqkvT,
                "woT": woT,
                "ropeA": ropeA,
                "ropeB": ropeB,
                "maskT": band,
                "ones": ones_col,
            }
        )
    return in_maps


def kernel(x, wq, wk, wv, wo, freqs_cos, freqs_sin, mask, start_pos=0):
    from concourse import bass_utils

    if "nc" not in _CACHE:
        _CACHE["nc"] = _build_nc()
    nc = _CACHE["nc"]

    in_maps = _prep_inputs(x, wq, wk, wv, wo, freqs_cos, freqs_sin, mask)
    res = bass_utils.run_bass_kernel_spmd(nc, in_maps, list(range(NCORES)))
    acc = np.zeros((DIM, BS), dtype=np.float64)
    for c in range(NCORES):
        acc += res.results[c]["outT"]
    return np.ascontiguousarray(acc.T).reshape(B, S, DIM).astype(np.float32)


# revision 4
# speedup vs baseline: 14.3535x; 14.3535x over previous
"""Tensor-parallel causal attention block for Trainium2 (8 NeuronCores).

Shapes (hardcoded): x (2, 2048, 4096), NH=32 heads of HD=128, fp32.
Sharding: tensor-parallel over heads -- each core owns 4 heads (wq/wk/wv
column-sharded, wo row-sharded); partial outputs are summed on the host.

Per-core kernel phases:
  1. fused QKV projection (f32r matmuls), RoPE fused into PSUM eviction
     using a host-side de-interleaving weight permutation + partition-swap.
  2. attention in transposed layout: scoresT = kT_blk.T-matmul, softmax
     along the partition (k) axis with the denominator computed by a
     ones-vector matmul; causal block skipping.
  3. output projection against the row-shard of wo -> partial outT.
"""
import sys

sys.path.insert(0, "/opt/trn_rl_repo")

import numpy as np

B, S, DIM, NH, HD = 2, 2048, 4096, 32, 128
NCORES = 8
HL = NH // NCORES          # 4 heads per core
BS = B * S                 # 4096 rows
P = 128
QT = 512                   # row-tile width (matmul moving dim)
NQT = BS // QT             # 8 row tiles
KO = DIM // P              # 32 contraction chunks
SCALE = 1.0 / np.sqrt(HD)
NEG = -30000.0             # "-inf" that survives exp() as exact 0

_CACHE: dict = {}


def _hrow(h, kind):
    """Row offset of head-h q/k/v inside the (1536, BS) qkvT intermediate."""
    half, idx = divmod(h, 2)
    return half * 768 + {"q": 0, "k": 256, "v": 512}[kind] + idx * P


def _build_nc():
    import concourse.mybir as mybir
    import concourse.tile as tile
    from concourse import bacc
    from concourse.masks import make_identity

    F32 = mybir.dt.float32
    F32R = mybir.dt.float32r
    AF = mybir.ActivationFunctionType
    OP = mybir.AluOpType

    nc = bacc.Bacc(trn_type="TRN2", target_bir_lowering=False, debug=False)

    xT = nc.dram_tensor("xT", [DIM, BS], F32, kind="ExternalInput").ap()
    wqkvT = nc.dram_tensor("wqkvT", [DIM, 12 * P], F32, kind="ExternalInput").ap()
    woT = nc.dram_tensor("woT", [HL * P, DIM], F32, kind="ExternalInput").ap()
    ropeA = nc.dram_tensor("ropeA", [P, BS], F32, kind="ExternalInput").ap()
    ropeB = nc.dram_tensor("ropeB", [P, BS], F32, kind="ExternalInput").ap()
    maskT = nc.dram_tensor("maskT", [QT, QT], F32, kind="ExternalInput").ap()
    ones = nc.dram_tensor("ones", [P, 1], F32, kind="ExternalInput").ap()
    outT = nc.dram_tensor("outT", [DIM, BS], F32, kind="ExternalOutput").ap()

    xT3 = xT.rearrange("(ko p) n -> p ko n", p=P)          # (128, 32, 4096)
    wqkvT3 = wqkvT.rearrange("(ko p) c -> p ko c", p=P)    # (128, 32, 1536)
    woT3 = woT.rearrange("(kc p) m -> p kc m", p=P)        # (128, 4, 4096)
    maskT3 = maskT.rearrange("(kb p) q -> p kb q", p=P)    # (128, 4, 512)

    with tile.TileContext(nc) as tc:
        with (
            nc.allow_low_precision(reason="f32r matmul pipeline"),
            tc.tile_pool(name="const", bufs=1) as cst,
            tc.tile_pool(name="dram", bufs=1, space="DRAM") as dpool,
        ):
            qkv_d = [
                dpool.tile([P, BS], F32R, tag=f"qkvd{g}", name=f"qkvd{g}")
                for g in range(12)
            ]

            ones_sb = cst.tile([P, 1], F32R)
            nc.sync.dma_start(ones_sb[:], ones.bitcast(F32R))
            ones_row = cst.tile([1, P], F32R)
            nc.sync.dma_start(ones_row[:], ones.rearrange("p o -> o p").bitcast(F32R))
            ident = cst.tile([P, P], F32)
            make_identity(nc, ident[:])

            # ---------------- Phase 1: fused QKV projection + RoPE ----------
            with (
                tc.tile_pool(name="p1w", bufs=1) as wpool,
                tc.tile_pool(name="p1x", bufs=4) as xpool,
                tc.tile_pool(name="rope", bufs=1) as rpool,
                tc.tile_pool(name="p1ev", bufs=3) as evpool,
                tc.tile_pool(name="p1ps", bufs=8, space="PSUM") as pspool,
            ):
                rA = rpool.tile([P, BS], F32)
                rB = rpool.tile([P, BS], F32)
                nc.sync.dma_start(rA[:], ropeA)
                nc.sync.dma_start(rB[:], ropeB)

                for half in range(2):
                    w_sb = wpool.tile([P, KO, 768], F32R, tag="w")
                    for sl in range(8):
                        ksl = slice(sl * 4, (sl + 1) * 4)
                        nc.sync.dma_start(
                            w_sb[:, ksl, :],
                            wqkvT3[:, ksl, half * 768 : (half + 1) * 768].bitcast(F32R),
                        )
                    for qt in range(NQT):
                        cols = slice(qt * QT, (qt + 1) * QT)
                        psums = [
                            pspool.tile([P, QT], F32, tag="pp", name=f"pp{half}_{qt}_{i}")
                            for i in range(6)
                        ]
                        for kc2 in range(KO // 2):
                            x_sb = xpool.tile([P, 2, QT], F32R, tag="x")
                            nc.sync.dma_start(
                                x_sb[:], xT3[:, 2 * kc2 : 2 * kc2 + 2, cols].bitcast(F32R)
                            )
                            for j in range(2):
                                kc = 2 * kc2 + j
                                for m in range(6):
                                    nc.tensor.matmul(
                                        psums[m][:],
                                        w_sb[:, kc, m * P : (m + 1) * P],
                                        x_sb[:, j, :],
                                        start=(kc == 0),
                                        stop=(kc == KO - 1),
                                        skip_group_check=True,
                                    )
                        for m in range(6):
                            dst = qkv_d[half * 6 + m][:, cols]
                            if m < 4:  # q or k head: fused RoPE eviction
                                t0 = evpool.tile([P, QT], F32, tag="t0")
                                if m % 2 == 0:
                                    nc.scalar.copy(t0[:], psums[m][:])
                                else:
                                    nc.vector.tensor_copy(out=t0[:], in_=psums[m][:])
                                ev1 = evpool.tile([P, QT], F32, tag="ev1")
                                nc.vector.tensor_tensor(
                                    ev1[:], t0[:], rA[:, cols], OP.mult
                                )
                                ev2 = evpool.tile([P, QT], F32, tag="ev2")
                                nc.vector.tensor_tensor(
                                    ev2[:], t0[:], rB[:, cols], OP.mult
                                )
                                ev2s = evpool.tile([P, QT], F32, tag="ev2s")
                                nc.gpsimd.dma_start(ev2s[0:64, :], ev2[64:128, :])
                                nc.gpsimd.dma_start(ev2s[64:128, :], ev2[0:64, :])
                                out_t = evpool.tile([P, QT], F32R, tag="evo")
                                nc.vector.tensor_tensor(
                                    out_t[:], ev1[:], ev2s[:], OP.add
                                )
                                nc.gpsimd.dma_start(dst, out_t[:])
                            else:  # v head: plain eviction
                                out_t = evpool.tile([P, QT], F32R, tag="evo")
                                if m % 2 == 0:
                                    nc.scalar.copy(out_t[:], psums[m][:])
                                else:
                                    nc.vector.tensor_copy(out=out_t[:], in_=psums[m][:])
                                nc.gpsimd.dma_start(dst, out_t[:])

            # ---------------- Phase 2: attention ----------------------------
            with tc.tile_pool(name="att", bufs=1) as attpool:
                attnT = attpool.tile([P, HL, BS], F32R)  # 64KB/partition

                with (
                    tc.tile_pool(name="bh", bufs=2) as bhpool,
                    tc.tile_pool(name="pr", bufs=18) as prpool,
                    tc.tile_pool(name="sm", bufs=3) as smpool,
                    tc.tile_pool(name="msk", bufs=1) as mpool,
                    tc.tile_pool(name="psS", bufs=3, space="PSUM") as psS,
                    tc.tile_pool(name="psO", bufs=2, space="PSUM") as psO,
                    tc.tile_pool(name="psD", bufs=1, space="PSUM") as psD,
                    tc.tile_pool(name="psT", bufs=2, space="PSUM") as psT,
                ):
                    mask_sb = mpool.tile([P, 4, QT], F32)
                    nc.sync.dma_start(mask_sb[:], maskT3)

                    for b in range(B):
                        bcols = slice(b * S, (b + 1) * S)
                        for h in range(HL):
                            qT_sb = bhpool.tile([P, S], F32R, tag="q")
                            kT_sb = bhpool.tile([P, S], F32R, tag="k")
                            vT_sb = bhpool.tile([P, S], F32R, tag="v")
                            gq = (h // 2) * 6 + (h % 2)
                            gk = (h // 2) * 6 + 2 + (h % 2)
                            gv = (h // 2) * 6 + 4 + (h % 2)
                            nc.sync.dma_start(qT_sb[:], qkv_d[gq][:, bcols])
                            nc.sync.dma_start(kT_sb[:], qkv_d[gk][:, bcols])
                            nc.sync.dma_start(vT_sb[:], qkv_d[gv][:, bcols])
                            # transpose V into (k-rows, d) blocks
                            v_bl = bhpool.tile([P, S // P, P], F32R, tag="vb")
                            for kb in range(S // P):
                                tp = psT.tile([P, P], F32, tag="tp")
                                nc.tensor.transpose(
                                    tp[:],
                                    vT_sb[:, kb * P : (kb + 1) * P].bitcast(F32),
                                    ident[:],
                                )
                                nc.scalar.copy(v_bl[:, kb, :], tp[:])

                            for jq in range(S // QT):
                                qsl = qT_sb[:, jq * QT : (jq + 1) * QT]
                                nkb = (jq + 1) * (QT // P)
                                probs = []
                                for kb in range(nkb):
                                    sP = psS.tile([P, QT], F32, tag="sP")
                                    nc.tensor.matmul(
                                        sP[:],
                                        kT_sb[:, kb * P : (kb + 1) * P],
                                        qsl,
                                        start=True,
                                        stop=True,
                                        skip_group_check=True,
                                    )
                                    if kb >= jq * (QT // P):  # diagonal band
                                        nc.vector.tensor_tensor(
                                            sP[:],
                                            sP[:],
                                            mask_sb[:, kb - jq * (QT // P), :],
                                            OP.add,
                                        )
                                    pr = prpool.tile([P, QT], F32R, tag="pr")
                                    nc.scalar.activation(
                                        pr[:], sP[:], AF.Exp, scale=SCALE
                                    )
                                    probs.append(pr)
                                outP = psO.tile([P, QT], F32, tag="outP")
                                for kb in range(nkb):
                                    nc.tensor.matmul(
                                        outP[:],
                                        v_bl[:, kb, :],
                                        probs[kb][:],
                                        start=(kb == 0),
                                        stop=(kb == nkb - 1),
                                        skip_group_check=True,
                                    )
                                denP = psD.tile([1, QT], F32, tag="denP")
                                for kb in range(nkb):
                                    nc.tensor.matmul(
                                        denP[:],
                                        ones_sb[:],
                                        probs[kb][:],
                                        start=(kb == 0),
                                        stop=(kb == nkb - 1),
                                        skip_group_check=True,
                                    )
                                rec = smpool.tile([1, QT], F32R, tag="rec")
                                nc.vector.reciprocal(rec[:], denP[:])
                                bcP = psT.tile([P, QT], F32, tag="tp")
                                nc.tensor.matmul(
                                    bcP[:],
                                    ones_row[:],
                                    rec[:],
                                    start=True,
                                    stop=True,
                                    skip_group_check=True,
                                )
                                rb = smpool.tile([P, QT], F32, tag="rb")
                                nc.scalar.copy(rb[:], bcP[:])
                                nc.vector.tensor_tensor(
                                    attnT[:, h, b * S + jq * QT : b * S + (jq + 1) * QT],
                                    outP[:],
                                    rb[:],
                                    OP.mult,
                                )

                # ---------------- Phase 3: output projection ----------------
                with (
                    tc.tile_pool(name="p3w", bufs=1) as wpool3,
                    tc.tile_pool(name="p3ev", bufs=4) as evpool3,
                    tc.tile_pool(name="p3ps", bufs=4, space="PSUM") as ps3,
                ):
                    wo_sb = wpool3.tile([P, HL, DIM], F32R)
                    for m in range(DIM // P):
                        nc.sync.dma_start(
                            wo_sb[:, :, m * P : (m + 1) * P],
                            woT3[:, :, m * P : (m + 1) * P].bitcast(F32R),
                        )
                    for qt in range(NQT):
                        cols = slice(qt * QT, (qt + 1) * QT)
                        for m in range(DIM // P):
                            oP = ps3.tile([P, QT], F32, tag="oP")
                            for kc in range(HL):
                                nc.tensor.matmul(
                                    oP[:],
                                    wo_sb[:, kc, m * P : (m + 1) * P],
                                    attnT[:, kc, cols],
                                    start=(kc == 0),
                                    stop=(kc == HL - 1),
                                    skip_group_check=True,
                                )
                            ev = evpool3.tile([P, QT], F32, tag="oev")
                            if m % 2 == 0:
                                nc.scalar.copy(ev[:], oP[:])
                            else:
                                nc.vector.tensor_copy(out=ev[:], in_=oP[:])
                            nc.gpsimd.dma_start(outT[m * P : (m + 1) * P, cols], ev[:])
    nc.compile()
    return nc


def _prep_inputs(x, wq, wk, wv, wo, freqs_cos, freqs_sin, mask):
    """Host-side shard prep. Returns per-core input maps."""
    x = np.asarray(x, dtype=np.float32)
    wq, wk, wv, wo = (np.asarray(a, dtype=np.float32) for a in (wq, wk, wv, wo))
    freqs_cos = np.asarray(freqs_cos, dtype=np.float32)
    freqs_sin = np.asarray(freqs_sin, dtype=np.float32)
    mask = np.asarray(mask, dtype=np.float32)

    xT = np.ascontiguousarray(x.reshape(BS, DIM).T)

    cosT = freqs_cos.T  # (64, S)
    sinT = freqs_sin.T
    ropeA = np.ascontiguousarray(
        np.tile(np.concatenate([cosT, cosT], axis=0), (1, B))
    ).astype(np.float32)
    ropeB = np.ascontiguousarray(
        np.tile(np.concatenate([sinT, -sinT], axis=0), (1, B))
    ).astype(np.float32)

    band = np.maximum(mask[:QT, :QT].T, NEG).astype(np.float32)
    band = np.ascontiguousarray(band)
    ones_col = np.ones((P, 1), dtype=np.float32)

    perm = np.concatenate([np.arange(0, HD, 2), np.arange(1, HD, 2)])

    in_maps = []
    for c in range(NCORES):
        heads = [c * HL + j for j in range(HL)]
        cols = []
        for half in range(2):
            hA, hB = heads[2 * half], heads[2 * half + 1]
            cols.append(wq[hA * HD : (hA + 1) * HD][perm].T)
            cols.append(wq[hB * HD : (hB + 1) * HD][perm].T)
            cols.append(wk[hA * HD : (hA + 1) * HD][perm].T)
            cols.append(wk[hB * HD : (hB + 1) * HD][perm].T)
            cols.append(wv[hA * HD : (hA + 1) * HD].T)
            cols.append(wv[hB * HD : (hB + 1) * HD].T)
        wqkvT = np.ascontiguousarray(np.concatenate(cols, axis=1))
        woT = np.ascontiguousarray(wo[:, c * HL * HD : (c + 1) * HL * HD].T)
        in_maps.append(
            {
                "xT": xT,
                "wqkvT": wqkvT,
                "woT": woT,
                "ropeA": ropeA,
                "ropeB": ropeB,
                "maskT": band,
                "ones": ones_col,
            }
        )
    return in_maps


def kernel(x, wq, wk, wv, wo, freqs_cos, freqs_sin, mask, start_pos=0):
    from concourse import bass_utils

    if "nc" not in _CACHE:
        _CACHE["nc"] = _build_nc()
    nc = _CACHE["nc"]

    in_maps = _prep_inputs(x, wq, wk, wv, wo, freqs_cos, freqs_sin, mask)
    res = bass_utils.run_bass_kernel_spmd(nc, in_maps, list(range(NCORES)))
    acc = np.zeros((DIM, BS), dtype=np.float64)
    for c in range(NCORES):
        acc += res.results[c]["outT"]
    return np.ascontiguousarray(acc.T).reshape(B, S, DIM).astype(np.float32)


# revision 16
# speedup vs baseline: 27.0705x; 1.8860x over previous
"""Tensor-parallel causal attention block for Trainium2 (8 NeuronCores).

Shapes (hardcoded): x (2, 2048, 4096), NH=32 heads of HD=128, fp32.
Sharding: tensor-parallel over heads -- each core owns 4 heads (wq/wk/wv
column-sharded, wo row-sharded); partial outputs are summed on the host.

Per-core kernel phases:
  1. fused QKV projection (f32r matmuls), RoPE fused into PSUM eviction
     using a host-side de-interleaving weight permutation + partition-swap.
  2. attention in transposed layout: scoresT = kT_blk.T-matmul, softmax
     along the partition (k) axis with the denominator computed by a
     ones-vector matmul; causal block skipping.
  3. output projection against the row-shard of wo -> partial outT.
"""
import sys

sys.path.insert(0, "/opt/trn_rl_repo")

import numpy as np

B, S, DIM, NH, HD = 2, 2048, 4096, 32, 128
NCORES = 8
HL = NH // NCORES          # 4 heads per core
BS = B * S                 # 4096 rows
P = 128
QT = 512                   # row-tile width (matmul moving dim)
NQT = BS // QT             # 8 row tiles
KO = DIM // P              # 32 contraction chunks
SCALE = 1.0 / np.sqrt(HD)
NEG = -30000.0             # "-inf" that survives exp() as exact 0

_CACHE: dict = {}
DEN_BATCH = False


def _hrow(h, kind):
    """Row offset of head-h q/k/v inside the (1536, BS) qkvT intermediate."""
    half, idx = divmod(h, 2)
    return half * 768 + {"q": 0, "k": 256, "v": 512}[kind] + idx * P


def _build_nc():
    import concourse.mybir as mybir
    import concourse.tile as tile
    from concourse import bacc, bass_isa
    from concourse.masks import make_identity

    F32 = mybir.dt.float32
    F32R = mybir.dt.float32r
    AF = mybir.ActivationFunctionType
    OP = mybir.AluOpType

    nc = bacc.Bacc(trn_type="TRN2", target_bir_lowering=False, debug=False)

    xT = nc.dram_tensor("xT", [DIM, BS], F32, kind="ExternalInput").ap()
    wqkvT = nc.dram_tensor("wqkvT", [DIM, 12 * P], F32, kind="ExternalInput").ap()
    woT = nc.dram_tensor("woT", [HL * P, DIM], F32, kind="ExternalInput").ap()
    ropeA = nc.dram_tensor("ropeA", [P, BS], F32, kind="ExternalInput").ap()
    ropeB = nc.dram_tensor("ropeB", [P, BS], F32, kind="ExternalInput").ap()
    maskT = nc.dram_tensor("maskT", [QT, QT], F32, kind="ExternalInput").ap()
    outT = nc.dram_tensor("outT", [DIM, BS], F32, kind="ExternalOutput").ap()

    xT3 = xT.rearrange("(ko p) n -> p ko n", p=P)          # (128, 32, 4096)
    wqkvT3 = wqkvT.rearrange("(ko p) c -> p ko c", p=P)    # (128, 32, 1536)
    woT3 = woT.rearrange("(kc p) m -> p kc m", p=P)        # (128, 4, 4096)
    maskT3 = maskT.rearrange("(kb p) q -> p kb q", p=P)    # (128, 4, 512)

    with tile.TileContext(nc) as tc:
        with (
            nc.allow_low_precision(reason="f32r matmul pipeline"),
            tc.tile_pool(name="const", bufs=1) as cst,
            tc.tile_pool(name="dram", bufs=1, space="DRAM") as dpool,
        ):
            qkv_d = [
                dpool.tile([P, BS], F32R, tag=f"qkvd{g}", name=f"qkvd{g}")
                for g in range(12)
            ]

            ident = cst.tile([P, P], F32)
            make_identity(nc, ident[:])

            # ---------------- Phase 1: fused QKV projection + RoPE ----------
            with (
                tc.tile_pool(name="p1w", bufs=1) as wpool,
                tc.tile_pool(name="p1x", bufs=8) as xpool,
                tc.tile_pool(name="rope", bufs=1) as rpool,
                tc.tile_pool(name="p1ev", bufs=3) as evpool,
                tc.tile_pool(name="p1ps", bufs=8, space="PSUM") as pspool,
            ):
                rA = rpool.tile([P, BS], F32)
                rB = rpool.tile([P, BS], F32)

                for half in range(2):
                    w_sb = wpool.tile([P, KO, 768], F32R, tag="w")

                    def emit_slab(sl, half=half, w_sb=w_sb):
                        ksl = slice(sl * 4, (sl + 1) * 4)
                        for m in range(6):
                            wc = half * 768 + m * P
                            nc.sync.dma_start(
                                w_sb[:, ksl, m * P : (m + 1) * P],
                                wqkvT3[:, ksl, wc : wc + P].bitcast(F32R),
                            )

                    emit_slab(0)
                    emit_slab(1)
                    for qt in range(NQT):
                        cols = slice(qt * QT, (qt + 1) * QT)
                        psums = [
                            pspool.tile([P, QT], F32, tag="pp", name=f"pp{half}_{qt}_{i}")
                            for i in range(6)
                        ]
                        for kc2 in range(KO // 2):
                            x_sb = xpool.tile([P, 2, QT], F32R, tag="x")
                            nc.sync.dma_start(
                                x_sb[:], xT3[:, 2 * kc2 : 2 * kc2 + 2, cols].bitcast(F32R)
                            )
                            if qt == 0 and kc2 in (2, 4, 6, 8, 10, 12):
                                emit_slab(kc2 // 2 + 1)
                            if half == 0 and qt == 0 and kc2 in (9, 10, 11, 12):
                                rch = kc2 - 9
                                rsl = slice(rch * (BS // 4), (rch + 1) * (BS // 4))
                                nc.sync.dma_start(rA[:, rsl], ropeA[:, rsl])
                                nc.sync.dma_start(rB[:, rsl], ropeB[:, rsl])
                            for j in range(2):
                                kc = 2 * kc2 + j
                                for m in range(6):
                                    nc.tensor.matmul(
                                        psums[m][:],
                                        w_sb[:, kc, m * P : (m + 1) * P],
                                        x_sb[:, j, :],
                                        start=(kc == 0),
                                        stop=(kc == KO - 1),
                                        skip_group_check=True,
                                    )
                        for m in range(6):
                            dst = qkv_d[half * 6 + m][:, cols]
                            if m < 4:  # q or k head: fused RoPE eviction
                                t0 = evpool.tile([P, QT], F32, tag="t0")
                                if m % 2 == 0:
                                    nc.scalar.copy(t0[:], psums[m][:])
                                else:
                                    nc.vector.tensor_copy(out=t0[:], in_=psums[m][:])
                                ev1 = evpool.tile([P, QT], F32, tag="ev1")
                                nc.vector.tensor_tensor(
                                    ev1[:], t0[:], rA[:, cols], OP.mult
                                )
                                ev2 = evpool.tile([P, QT], F32, tag="ev2")
                                nc.vector.tensor_tensor(
                                    ev2[:], t0[:], rB[:, cols], OP.mult
                                )
                                ev2s = evpool.tile([P, QT], F32, tag="ev2s")
                                nc.gpsimd.dma_start(ev2s[0:64, :], ev2[64:128, :])
                                nc.gpsimd.dma_start(ev2s[64:128, :], ev2[0:64, :])
                                out_t = evpool.tile([P, QT], F32R, tag="evo")
                                nc.vector.tensor_tensor(
                                    out_t[:], ev1[:], ev2s[:], OP.add
                                )
                                nc.sync.dma_start(dst, out_t[:])
                            else:  # v head: plain eviction
                                out_t = evpool.tile([P, QT], F32R, tag="evo")
                                if m % 2 == 0:
                                    nc.scalar.copy(out_t[:], psums[m][:])
                                else:
                                    nc.vector.tensor_copy(out=out_t[:], in_=psums[m][:])
                                nc.sync.dma_start(dst, out_t[:])

            # ---------------- Phase 2: attention ----------------------------
            with (
                tc.tile_pool(name="att", bufs=1) as attpool,
                tc.tile_pool(name="p3w", bufs=4) as wpool3,
                tc.tile_pool(name="p3ps", bufs=2, space="PSUM") as ps3,
            ):
                attnT = attpool.tile([P, HL, BS], F32R)  # 64KB/partition

                with (
                    tc.tile_pool(name="bh", bufs=2) as bhpool,
                    tc.tile_pool(name="pr", bufs=8) as prpool,
                    tc.tile_pool(name="sm", bufs=3) as smpool,
                    tc.tile_pool(name="msk", bufs=1) as mpool,
                    tc.tile_pool(name="psS", bufs=4, space="PSUM") as psS,
                    tc.tile_pool(name="psO", bufs=2, space="PSUM") as psO,
                ):
                    mask_sb = mpool.tile([P, 4, QT], F32)
                    nc.sync.dma_start(mask_sb[:], maskT3)
                    att_markers = []

                    for b in range(B):
                        bcols = slice(b * S, (b + 1) * S)
                        for h in range(HL):
                            qT_sb = bhpool.tile([P, S], F32R, tag="q")
                            kT_sb = bhpool.tile([P, S], F32R, tag="k")
                            vT_sb = bhpool.tile([P, S], F32R, tag="v")
                            gq = (h // 2) * 6 + (h % 2)
                            gk = (h // 2) * 6 + 2 + (h % 2)
                            gv = (h // 2) * 6 + 4 + (h % 2)
                            for ch in range(4):
                                cs = slice(ch * (S // 4), (ch + 1) * (S // 4))
                                gcs = slice(b * S + ch * (S // 4), b * S + (ch + 1) * (S // 4))
                                mk = nc.sync.dma_start(qT_sb[:, cs], qkv_d[gq][:, gcs])
                                if h == 0 and ch == 0:
                                    att_markers.append(mk)
                                nc.sync.dma_start(kT_sb[:, cs], qkv_d[gk][:, gcs])
                                nc.sync.dma_start(vT_sb[:, cs], qkv_d[gv][:, gcs])
                            # transpose V into (k-rows, d) blocks
                            v_bl = mpool.tile([P, S // P, P], F32R, tag="vb", name=f"vb{b}_{h}")
                            for kb in range(S // P):
                                tp = psS.tile([P, P], F32, tag="sP", name=f"tp{b}_{h}_{kb}")
                                nc.tensor.transpose(
                                    tp[:],
                                    vT_sb[:, kb * P : (kb + 1) * P].bitcast(F32),
                                    ident[:],
                                )
                                nc.scalar.copy(v_bl[:, kb, :], tp[:])

                            for jq in range(S // QT):
                                qsl = qT_sb[:, jq * QT : (jq + 1) * QT]
                                nkb = (jq + 1) * (QT // P)
                                outP = psO.tile([P, QT], F32, tag="outP")
                                acc = smpool.tile([P, QT], F32, tag="acc",
                                                  name=f"acc{b}_{h}_{jq}")
                                for kb in range(nkb):
                                    sP = psS.tile([P, QT], F32, tag="sP")
                                    nc.tensor.matmul(
                                        sP[:],
                                        kT_sb[:, kb * P : (kb + 1) * P],
                                        qsl,
                                        start=True,
                                        stop=True,
                                        skip_group_check=True,
                                    )
                                    if kb >= jq * (QT // P):  # diagonal band
                                        nc.vector.tensor_tensor(
                                            sP[:],
                                            sP[:],
                                            mask_sb[:, kb - jq * (QT // P), :],
                                            OP.add,
                                        )
                                    pr = prpool.tile([P, QT], F32R, tag="pr")
                                    nc.scalar.activation(
                                        pr[:], sP[:], AF.Exp, scale=SCALE
                                    )
                                    nc.tensor.matmul(
                                        outP[:],
                                        v_bl[:, kb, :],
                                        pr[:],
                                        start=(kb == 0),
                                        stop=(kb == nkb - 1),
                                        skip_group_check=True,
                                    )
                                    if kb == 0:
                                        nc.vector.tensor_copy(
                                            out=acc[:], in_=pr[:].bitcast(F32)
                                        )
                                    else:
                                        nc.vector.tensor_tensor(
                                            acc[:], acc[:], pr[:].bitcast(F32), OP.add
                                        )
                                den_bc = smpool.tile([P, QT], F32, tag="den",
                                                     name=f"den{b}_{h}_{jq}")
                                nc.gpsimd.partition_all_reduce(
                                    den_bc[:], acc[:], channels=P,
                                    reduce_op=bass_isa.ReduceOp.add,
                                )
                                rec = smpool.tile([P, QT], F32, tag="rec")
                                nc.vector.reciprocal(rec[:], den_bc[:])
                                nc.vector.tensor_tensor(
                                    attnT[:, h, b * S + jq * QT : b * S + (jq + 1) * QT],
                                    outP[:],
                                    rec[:],
                                    OP.mult,
                                )

                # ---------------- Phase 3: output projection ----------------
                with (
                    tc.tile_pool(name="p3ev", bufs=4) as evpool3,
                ):
                    for bh3 in range(B):
                        for m in range(DIM // P):
                            woc = wpool3.tile([P, HL, P], F32R, tag="woc",
                                              name=f"woc{bh3}_{m}")
                            wdma = nc.sync.dma_start(
                                woc[:], woT3[:, :, m * P : (m + 1) * P].bitcast(F32R)
                            )
                            from concourse.tile_rust import add_dep_helper
                            add_dep_helper(
                                wdma.ins, att_markers[bh3].ins, sync=False,
                                reason="delay wo load until this batch's attention starts",
                            )
                            for qt3 in range(NQT // B):
                                qt = bh3 * (NQT // B) + qt3
                                cols = slice(qt * QT, (qt + 1) * QT)
                                oP = ps3.tile([P, QT], F32, tag="oP")
                                for kc in range(HL):
                                    nc.tensor.matmul(
                                        oP[:],
                                        woc[:, kc, :],
                                        attnT[:, kc, cols],
                                        start=(kc == 0),
                                        stop=(kc == HL - 1),
                                        skip_group_check=True,
                                    )
                                ev = evpool3.tile([P, QT], F32, tag="oev")
                                if m % 2 == 0:
                                    nc.scalar.copy(ev[:], oP[:])
                                else:
                                    nc.vector.tensor_copy(out=ev[:], in_=oP[:])
                                nc.sync.dma_start(outT[m * P : (m + 1) * P, cols], ev[:])
    nc.compile()
    return nc


def _prep_inputs(x, wq, wk, wv, wo, freqs_cos, freqs_sin, mask):
    """Host-side shard prep. Returns per-core input maps."""
    x = np.asarray(x, dtype=np.float32)
    wq, wk, wv, wo = (np.asarray(a, dtype=np.float32) for a in (wq, wk, wv, wo))
    freqs_cos = np.asarray(freqs_cos, dtype=np.float32)
    freqs_sin = np.asarray(freqs_sin, dtype=np.float32)
    mask = np.asarray(mask, dtype=np.float32)

    xT = np.ascontiguousarray(x.reshape(BS, DIM).T)

    cosT = freqs_cos.T  # (64, S)
    sinT = freqs_sin.T
    ropeA = np.ascontiguousarray(
        np.tile(np.concatenate([cosT, cosT], axis=0), (1, B))
    ).astype(np.float32)
    ropeB = np.ascontiguousarray(
        np.tile(np.concatenate([sinT, -sinT], axis=0), (1, B))
    ).astype(np.float32)

    band = np.maximum(mask[:QT, :QT].T, NEG).astype(np.float32)
    band = np.ascontiguousarray(band)

    perm = np.concatenate([np.arange(0, HD, 2), np.arange(1, HD, 2)])

    in_maps = []
    for c in range(NCORES):
        heads = [c * HL + j for j in range(HL)]
        cols = []
        for half in range(2):
            hA, hB = heads[2 * half], heads[2 * half + 1]
            cols.append(wq[hA * HD : (hA + 1) * HD][perm].T)
            cols.append(wq[hB * HD : (hB + 1) * HD][perm].T)
            cols.append(wk[hA * HD : (hA + 1) * HD][perm].T)
            cols.append(wk[hB * HD : (hB + 1) * HD][perm].T)
            cols.append(wv[hA * HD : (hA + 1) * HD].T)
            cols.append(wv[hB * HD : (hB + 1) * HD].T)
        wqkvT = np.ascontiguousarray(np.concatenate(cols, axis=1))
        woT = np.ascontiguousarray(wo[:, c * HL * HD : (c + 1) * HL * HD].T)
        in_maps.append(
            {
                "xT": xT,
                "wqkvT": wqkvT,
                "woT": woT,
                "ropeA": ropeA,
                "ropeB": ropeB,
                "maskT": band,
            }
        )
    return in_maps


def kernel(x, wq, wk, wv, wo, freqs_cos, freqs_sin, mask, start_pos=0):
    from concourse import bass_utils

    if "nc" not in _CACHE:
        _CACHE["nc"] = _build_nc()
    nc = _CACHE["nc"]

    in_maps = _prep_inputs(x, wq, wk, wv, wo, freqs_cos, freqs_sin, mask)
    res = bass_utils.run_bass_kernel_spmd(nc, in_maps, list(range(NCORES)))
    acc = np.zeros((DIM, BS), dtype=np.float64)
    for c in range(NCORES):
        acc += res.results[c]["outT"]
    return np.ascontiguousarray(acc.T).reshape(B, S, DIM).astype(np.float32)
